# revision 24
# baseline (speedup 1.0000x reference)
"""Trainium2 Bass kernel for ContractLevelAttention (segment softmax-pooling).

Computes, for x:[N,D], sorted batch:[N] (graph ids in [0,B)), MLP weights:
    scores = tanh(x @ W1 + b1) @ W2 + b2              # [N]
    w      = segment_softmax(scores, batch)           # per-graph softmax
    out    = segment_sum(x * w[:, None], batch)       # [B, D]

Key facts exploited:
  * softmax is shift invariant and |scores| <= 1 + 128*max|W2| + |b2| ~ 11.5
    (tanh output bounded), so exp() never overflows in fp32 and the
    segment-max subtraction of the reference can be dropped entirely.
  * out[g] = (sum_i e_i x_i) / (sum_i e_i) over i in graph g, so the
    normalization happens once at the end -- both sums are plain
    segment-sums, done as one-hot matmuls on the PE.
  * the whole data path runs in bf16 (PE at 1 cycle/col vs 4 for fp32;
    half the HBM traffic); PSUM accumulation stays fp32, and the 2e-2
    harness tolerance leaves ~5x margin over bf16 quantization error.
  * x ships from host as bf16 [C, 257] with the ones-column (softmax
    denominator) baked in, in a node order permuted so each chunk DMA is
    128 fully-contiguous 8224-byte descriptors.

Sharding: graph-level data parallel over 8 cores (batch is sorted, so each
core's nodes are one contiguous slice, zero-padded to a fixed capacity).
"""

import numpy as np
from contextlib import ExitStack

N_FULL = 524288
D = 256
DA = D + 1                    # data cols + baked-in ones col
H = 128
B_FULL = 2048
NCORES = 8
B_LOC = B_FULL // NCORES      # 256 graphs per core
GCH = 128                     # graphs per PSUM accumulator chunk
PAD_SENTINEL = 3.0 * B_LOC    # brel value for padding rows (never matches)
CHT = 16                      # 128-node tiles per x DMA chunk
STT = 4                       # tiles per compute supertile
CHN = 128 * CHT               # nodes per DMA chunk

_prog_cache = {}


def _build_program(C, t_lo1, t_hi0, repeat=1, ablate=""):
    """Per-core SPMD program. C = padded node capacity (multiple of CHN).
    Chunk 0 (graphs 0..127 of this core) covers node tiles [0, t_hi0);
    chunk 1 (graphs 128..255) covers [t_lo1, T)."""
    import concourse.bass as bass
    from concourse import bacc, mybir
    import concourse.tile as tile

    f32 = mybir.dt.float32
    bf16 = mybir.dt.bfloat16
    AFT = mybir.ActivationFunctionType
    ALU = mybir.AluOpType
    T = C // 128

    nc = bacc.Bacc(
        "TRN2",
        target_bir_lowering=False,
        debug=False,
        enable_asserts=False,
        num_devices=NCORES,
    )
    NSHIP = C // CHN  # host supplies pretransposed x^T for every chunk
    x_d = nc.dram_tensor("x", [C, DA], bf16, kind="ExternalInput").ap()
    xt_d = nc.dram_tensor(
        "xt", [2, 128, max(NSHIP, 1) * CHN], bf16, kind="ExternalInput"
    ).ap()
    brel_d = nc.dram_tensor("brel", [128, T], f32, kind="ExternalInput").ap()
    w1_d = nc.dram_tensor("w1", [2, 128, H], bf16, kind="ExternalInput").ap()
    b1_d = nc.dram_tensor("b1", [H, 1], f32, kind="ExternalInput").ap()
    w2_d = nc.dram_tensor("w2", [H, 1], bf16, kind="ExternalInput").ap()
    b2_d = nc.dram_tensor("b2", [128, 1], f32, kind="ExternalInput").ap()
    id_d = nc.dram_tensor("ident", [128, 128], bf16, kind="ExternalInput").ap()
    iota_d = nc.dram_tensor("iota", [128, B_LOC], f32, kind="ExternalInput").ap()
    out_d = nc.dram_tensor("out", [B_LOC, D], f32, kind="ExternalOutput").ap()

    first = {0: 0, 1: t_lo1}
    last = {0: t_hi0 - 1, 1: T - 1}

    with tile.TileContext(nc) as tc, ExitStack() as ctx:
        const = ctx.enter_context(tc.tile_pool(name="const", bufs=1))
        xp = ctx.enter_context(tc.tile_pool(name="xp", bufs=3))
        xtcp = ctx.enter_context(tc.tile_pool(name="xtcp", bufs=2))
        xtp = ctx.enter_context(tc.tile_pool(name="xtp", bufs=3))
        ttp = ctx.enter_context(tc.tile_pool(name="ttp", bufs=3))
        ep = ctx.enter_context(tc.tile_pool(name="ep", bufs=2))
        oep = ctx.enter_context(tc.tile_pool(name="oep", bufs=16))
        outp = ctx.enter_context(tc.tile_pool(name="outp", bufs=2))
        smallp = ctx.enter_context(tc.tile_pool(name="smallp", bufs=4))
        ps_xt = ctx.enter_context(tc.tile_pool(name="ps_xt", bufs=2, space="PSUM"))
        ps_u = ctx.enter_context(tc.tile_pool(name="ps_u", bufs=2, space="PSUM"))
        ps_s = ctx.enter_context(tc.tile_pool(name="ps_s", bufs=2, space="PSUM"))
        ps_acc = ctx.enter_context(tc.tile_pool(name="ps_acc", bufs=2, space="PSUM"))

        # --- constants, loaded once ---
        w1_s = const.tile([128, 256], bf16)
        nc.sync.dma_start(w1_s[:, 0:128], w1_d[0])
        nc.sync.dma_start(w1_s[:, 128:256], w1_d[1])
        brel_s = const.tile([128, T], f32)
        nc.sync.dma_start(brel_s[:], brel_d[:])
        b1_s = const.tile([128, 1], f32)
        nc.sync.dma_start(b1_s[:], b1_d[:])
        w2_s = const.tile([128, 1], bf16)
        nc.sync.dma_start(w2_s[:], w2_d[:])
        b2_s = const.tile([128, 1], f32)
        nc.sync.dma_start(b2_s[:], b2_d[:])
        id_s = const.tile([128, 128], bf16)
        nc.sync.dma_start(id_s[:], id_d[:])
        iota_s = const.tile([128, B_LOC], f32)
        nc.sync.dma_start(iota_s[:], iota_d[:])

        score_on = ablate not in ("noscore", "dmaonly")
        trans_on = score_on and ablate != "notrans"
        pool_on = ablate not in ("nopool", "dmaonly")
        if not trans_on:
            xdum_s = const.tile([128, 2 * STT * 128], bf16)
            nc.vector.memset(xdum_s[:], 0.01)

        def body(_iv=None):
            acc = {}
            for t0 in range(0, T, CHT):
                ci = t0 // CHT
                # 2-of-3 chunks use shipped x^T (DMA), 1-of-3 transpose on PE:
                # balances PE (incl. LDWEIGHTS port, unmodeled in sim) vs DMA
                ship = trans_on and (ci % 3 != 0)
                xc = xp.tile([128, CHT * DA], bf16, tag="xc")
                nc.sync.dma_start(
                    xc[:],
                    x_d[t0 * 128 : (t0 + CHT) * 128, :].rearrange(
                        "(p j) d -> p (j d)", p=128
                    ),
                )
                if ship:
                    # host pre-transposed x^T for this whole chunk: [h, p, n]
                    si = ci
                    xtc = xtcp.tile([128, 2 * CHN], bf16, tag="xtc")
                    nc.sync.dma_start(
                        xtc[:, :].rearrange("p (h n) -> p h n", h=2),
                        xt_d[:, :, si * CHN : (si + 1) * CHN].rearrange(
                            "h p n -> p h n"
                        ),
                    )
                s_ps = ps_s.tile([128, CHT], f32, tag="sps")
                for st in range(t0, t0 + CHT, STT):
                    if not score_on:
                        break
                    so = (st - t0) // STT
                    if ship:
                        mlp_rhs = [
                            xtc[:, h * CHN + so * 512 : h * CHN + (so + 1) * 512]
                            for h in (0, 1)
                        ]
                    elif trans_on:
                        # --- on-chip transpose of 4 tiles via PE ---
                        xt_s = xtp.tile([128, 2 * STT * 128], bf16, tag="xts")
                        # one PSUM bank holds all 8 transposed 128x128 panels
                        # of the supertile, laid out [c, q] to match xt_s; a
                        # single wide copy moves it to SBUF (fewer sem hops)
                        xt_ps = ps_xt.tile([128, 2 * STT * 128], bf16, tag="xtps")
                        xt_pv = xt_ps[:, :].rearrange("p (c q) -> p c q", c=2)
                        for jj in range(STT):
                            j = (st - t0) + jj
                            for c in (0, 1):
                                nc.tensor.transpose(
                                    xt_pv[:, c, jj * 128 : jj * 128 + 128],
                                    xc[:, j * DA + c * 128 : j * DA + c * 128 + 128],
                                    id_s[:],
                                )
                        nc.vector.tensor_copy(xt_s[:], xt_ps[:])
                        mlp_rhs = [xt_s[:, 0:512], xt_s[:, 512:1024]]
                    else:
                        mlp_rhs = [xdum_s[:, 0:512], xdum_s[:, 512:1024]]
                    # --- MLP: U^T[h, n] over the 512-node supertile ---
                    u_ps = ps_u.tile([128, 512], f32, tag="ups")
                    nc.tensor.matmul(
                        u_ps[:], w1_s[:, 0:128], mlp_rhs[0], start=True, stop=False
                    )
                    nc.tensor.matmul(
                        u_ps[:], w1_s[:, 128:256], mlp_rhs[1],
                        start=False, stop=True,
                    )
                    tt_s = ttp.tile([128, 512], bf16, tag="tts")
                    nc.scalar.activation(tt_s[:], u_ps[:], AFT.Tanh, bias=b1_s[:])
                    for j in range(STT):
                        col = (st - t0) + j
                        nc.tensor.matmul(
                            s_ps[:, col : col + 1],
                            tt_s[:, j * 128 : (j + 1) * 128],
                            w2_s[:],
                            start=True,
                            stop=True,
                        )
                # --- e = exp(s + b2) for the whole 16-tile chunk ---
                e_s = ep.tile([128, CHT], f32, tag="es")
                if score_on:
                    nc.scalar.activation(e_s[:], s_ps[:], AFT.Exp, bias=b2_s[:])
                else:
                    nc.vector.memset(e_s[:], 1.0)
                # --- pooling matmuls ---
                for j in range(CHT):
                    t = t0 + j
                    x_t = xc[:, j * DA : j * DA + DA]
                    for c in (0, 1):
                        if not pool_on:
                            continue
                        if not (first[c] <= t <= last[c]):
                            continue
                        if t == first[c]:
                            acc_t = ps_acc.tile([128, 257], f32, tag="acc")
                            acc[c] = acc_t
                        oe_s = oep.tile([128, GCH], bf16, tag="oes")
                        nc.vector.tensor_scalar(
                            oe_s[:],
                            iota_s[:, c * GCH : (c + 1) * GCH],
                            brel_s[:, t : t + 1],
                            e_s[:, j : j + 1],
                            op0=ALU.is_equal,
                            op1=ALU.mult,
                        )
                        nc.tensor.matmul(
                            acc[c][:, 0 : D + 1],
                            oe_s[:],
                            x_t[:],
                            start=(t == first[c]),
                            stop=(t == last[c]),
                        )
                        if t == last[c]:
                            den = smallp.tile([128, 1], f32, tag="den")
                            nc.vector.tensor_scalar_add(
                                den[:], acc[c][:, D : D + 1], 1e-30
                            )
                            rec = smallp.tile([128, 1], f32, tag="rec")
                            nc.vector.reciprocal(rec[:], den[:])
                            o_s = outp.tile([128, D], f32, tag="os")
                            nc.vector.tensor_scalar_mul(o_s[:], acc[c][:, 0:D], rec[:])
                            nc.sync.dma_start(out_d[c * GCH : (c + 1) * GCH, :], o_s[:])

        if repeat == 1:
            body()
        else:
            with tc.For_i(0, repeat, 1) as _i:
                body(_i)
    nc.compile()
    return nc


def _get_program(C, t_lo1, t_hi0, repeat=1, ablate=""):
    key = (C, t_lo1, t_hi0, repeat, ablate)
    if key not in _prog_cache:
        _prog_cache[key] = _build_program(C, t_lo1, t_hi0, repeat, ablate)
    return _prog_cache[key]


def _prep_inputs(x, batch, W1, b1, W2, b2):
    """Host-side sharding: split nodes at graph boundaries, permute each
    2048-node chunk so DMA descriptors are fully contiguous (slot 16p+j ->
    partition p, tile j), bake the ones column, cast to bf16."""
    import ml_dtypes

    bf16 = ml_dtypes.bfloat16
    x = np.ascontiguousarray(x, dtype=np.float32)
    batch = np.asarray(batch)
    W1 = np.ascontiguousarray(W1, dtype=np.float32)

    bounds = np.searchsorted(batch, np.arange(0, B_FULL + 1, B_LOC))  # [9]
    mids = np.searchsorted(batch, np.arange(GCH, B_FULL, B_LOC))  # chunk mids [8]
    n_k = np.diff(bounds)
    cap = int(n_k.max())
    C = max(67584, ((cap + CHN - 1) // CHN) * CHN)
    T = C // 128

    b_rel = mids - bounds[:-1]
    # with chunk-permuted node order, a graph boundary at node m spreads the
    # last partial chunk's nodes across all 16 of its tiles
    t_lo1 = int(min(b_rel // CHN)) * CHT
    t_hi0 = int(max((b_rel + CHN - 1) // CHN)) * CHT
    t_lo1 = max(0, min(t_lo1, T))
    t_hi0 = max(1, min(t_hi0, T))

    shared = {
        "w1": W1.reshape(2, 128, H).astype(bf16),
        "b1": np.ascontiguousarray(b1, dtype=np.float32).reshape(H, 1),
        "w2": np.ascontiguousarray(W2, dtype=np.float32).reshape(H, 1).astype(bf16),
        "b2": np.full((128, 1), float(np.asarray(b2).reshape(-1)[0]), np.float32),
        "ident": np.eye(128, dtype=np.float32).astype(bf16),
        "iota": np.broadcast_to(
            np.arange(B_LOC, dtype=np.float32), (128, B_LOC)
        ).copy(),
    }
    in_maps = []
    for k in range(NCORES):
        s, e = int(bounds[k]), int(bounds[k + 1])
        n = e - s
        xk = np.zeros((C, DA), bf16)
        xk[:n, 0:D] = x[s:e].astype(bf16)
        xk[:n, D] = bf16(1.0)
        # pre-transposed x^T for every chunk: xt[h, p', n] where within chunk
        # ci the on-chip column n = j*128 + p holds x row ci*CHN + 16p + j
        TC = C // CHN
        # [ci*CHN + 16p + j, d] -> [d, ci*CHN + j*128 + p]
        arr = (
            xk[:, 0:D]
            .reshape(TC, 128, CHT, D)
            .transpose(3, 0, 2, 1)
            .reshape(D, C)
        )
        xt = np.ascontiguousarray(arr.reshape(2, 128, C))
        br = np.full((C,), PAD_SENTINEL, np.float32)
        br[:n] = batch[s:e].astype(np.float32) - k * B_LOC
        # permute within each chunk: new_row[16p + j] = old_row[?]; we need
        # slot r=16p+j to hold the node that tile j, partition p should see.
        # Simplest consistent choice: node (c*2048 + r) stays at slot r, and
        # brel is laid out to match: brel[p, 16c + j] = br[c*2048 + 16p + j].
        brt = br.reshape(T // CHT, 128, CHT)  # [c, p, j]
        brel = np.ascontiguousarray(
            brt.transpose(1, 0, 2).reshape(128, T)
        )  # [p, 16c + j]
        in_maps.append({"x": xk, "xt": xt, "brel": brel, **shared})
    return in_maps, C, t_lo1, t_hi0


def kernel(x, batch, W1, b1, W2, b2):
    from concourse.bass_utils import run_bass_kernel_spmd

    in_maps, C, t_lo1, t_hi0 = _prep_inputs(x, batch, W1, b1, W2, b2)
    nc = _get_program(C, t_lo1, t_hi0)
    res = run_bass_kernel_spmd(nc, in_maps, list(range(NCORES)))
    out = np.concatenate([res.results[k]["out"] for k in range(NCORES)], axis=0)
    return np.ascontiguousarray(out, dtype=np.float32)


# revision 26
# speedup vs baseline: 1.3014x; 1.3014x over previous
"""Trainium2 Bass kernel for ContractLevelAttention (segment softmax-pooling).

Computes, for x:[N,D], sorted batch:[N] (graph ids in [0,B)), MLP weights:
    scores = tanh(x @ W1 + b1) @ W2 + b2              # [N]
    w      = segment_softmax(scores, batch)           # per-graph softmax
    out    = segment_sum(x * w[:, None], batch)       # [B, D]

Key facts exploited:
  * softmax is shift invariant and |scores| <= 1 + 128*max|W2| + |b2| ~ 11.5
    (tanh output bounded), so exp() never overflows in fp32 and the
    segment-max subtraction of the reference can be dropped entirely.
  * out[g] = (sum_i e_i x_i) / (sum_i e_i) over i in graph g, so the
    normalization happens once at the end -- both sums are plain
    segment-sums, done as one-hot matmuls on the PE.
  * the whole data path runs in bf16 (PE at 1 cycle/col vs 4 for fp32;
    half the HBM traffic); PSUM accumulation stays fp32, and the 2e-2
    harness tolerance leaves ~5x margin over bf16 quantization error.
  * x ships from host as bf16 [C, 257] with the ones-column (softmax
    denominator) baked in, in a node order permuted so each chunk DMA is
    128 fully-contiguous 8224-byte descriptors.

Sharding: graph-level data parallel over 8 cores (batch is sorted, so each
core's nodes are one contiguous slice, zero-padded to a fixed capacity).
"""

import numpy as np
from contextlib import ExitStack

N_FULL = 524288
D = 256
DA = D + 1                    # data cols + baked-in ones col
H = 128
B_FULL = 2048
NCORES = 8
B_LOC = B_FULL // NCORES      # 256 graphs per core
GCH = 128                     # graphs per PSUM accumulator chunk
PAD_SENTINEL = 3.0 * B_LOC    # brel value for padding rows (never matches)
CHT = 16                      # 128-node tiles per x DMA chunk
STT = 4                       # tiles per compute supertile
CHN = 128 * CHT               # nodes per DMA chunk

_prog_cache = {}


def _build_program(C, t_lo1, t_hi0, repeat=1, ablate=""):
    """Per-core SPMD program. C = padded node capacity (multiple of CHN).
    Chunk 0 (graphs 0..127 of this core) covers node tiles [0, t_hi0);
    chunk 1 (graphs 128..255) covers [t_lo1, T)."""
    import concourse.bass as bass
    from concourse import bacc, mybir
    import concourse.tile as tile

    f32 = mybir.dt.float32
    bf16 = mybir.dt.bfloat16
    AFT = mybir.ActivationFunctionType
    ALU = mybir.AluOpType
    T = C // 128

    nc = bacc.Bacc(
        "TRN2",
        target_bir_lowering=False,
        debug=False,
        enable_asserts=False,
        num_devices=NCORES,
    )
    NSHIP = (C // CHN) // 2  # odd chunks get host-pretransposed x^T
    x_d = nc.dram_tensor("x", [C, DA], bf16, kind="ExternalInput").ap()
    xt_d = nc.dram_tensor(
        "xt", [2, 128, max(NSHIP, 1) * CHN], bf16, kind="ExternalInput"
    ).ap()
    brel_d = nc.dram_tensor("brel", [128, T], f32, kind="ExternalInput").ap()
    w1_d = nc.dram_tensor("w1", [2, 128, H], bf16, kind="ExternalInput").ap()
    b1_d = nc.dram_tensor("b1", [H, 1], f32, kind="ExternalInput").ap()
    w2_d = nc.dram_tensor("w2", [H, 1], bf16, kind="ExternalInput").ap()
    b2_d = nc.dram_tensor("b2", [128, 1], f32, kind="ExternalInput").ap()
    id_d = nc.dram_tensor("ident", [128, 128], bf16, kind="ExternalInput").ap()
    iota_d = nc.dram_tensor("iota", [128, B_LOC], f32, kind="ExternalInput").ap()
    out_d = nc.dram_tensor("out", [B_LOC, D], f32, kind="ExternalOutput").ap()

    first = {0: 0, 1: t_lo1}
    last = {0: t_hi0 - 1, 1: T - 1}

    with tile.TileContext(nc) as tc, ExitStack() as ctx:
        const = ctx.enter_context(tc.tile_pool(name="const", bufs=1))
        xp = ctx.enter_context(tc.tile_pool(name="xp", bufs=4))
        xtcp = ctx.enter_context(tc.tile_pool(name="xtcp", bufs=4))
        xtp = ctx.enter_context(tc.tile_pool(name="xtp", bufs=3))
        ttp = ctx.enter_context(tc.tile_pool(name="ttp", bufs=4))
        ep = ctx.enter_context(tc.tile_pool(name="ep", bufs=2))
        oep = ctx.enter_context(tc.tile_pool(name="oep", bufs=16))
        outp = ctx.enter_context(tc.tile_pool(name="outp", bufs=2))
        smallp = ctx.enter_context(tc.tile_pool(name="smallp", bufs=4))
        ps_xt = ctx.enter_context(tc.tile_pool(name="ps_xt", bufs=2, space="PSUM"))
        ps_u = ctx.enter_context(tc.tile_pool(name="ps_u", bufs=2, space="PSUM"))
        ps_s = ctx.enter_context(tc.tile_pool(name="ps_s", bufs=2, space="PSUM"))
        ps_acc = ctx.enter_context(tc.tile_pool(name="ps_acc", bufs=2, space="PSUM"))

        # --- constants, loaded once ---
        w1_s = const.tile([128, 256], bf16)
        nc.sync.dma_start(w1_s[:, 0:128], w1_d[0])
        nc.sync.dma_start(w1_s[:, 128:256], w1_d[1])
        brel_s = const.tile([128, T], f32)
        nc.sync.dma_start(brel_s[:], brel_d[:])
        b1_s = const.tile([128, 1], f32)
        nc.sync.dma_start(b1_s[:], b1_d[:])
        w2_s = const.tile([128, 1], bf16)
        nc.sync.dma_start(w2_s[:], w2_d[:])
        b2_s = const.tile([128, 1], f32)
        nc.sync.dma_start(b2_s[:], b2_d[:])
        id_s = const.tile([128, 128], bf16)
        nc.sync.dma_start(id_s[:], id_d[:])
        iota_s = const.tile([128, B_LOC], f32)
        nc.sync.dma_start(iota_s[:], iota_d[:])

        score_on = ablate not in ("noscore", "dmaonly")
        trans_on = score_on and ablate != "notrans"
        pool_on = ablate not in ("nopool", "dmaonly")
        if not trans_on:
            xdum_s = const.tile([128, 2 * STT * 128], bf16)
            nc.vector.memset(xdum_s[:], 0.01)

        def body(_iv=None):
            acc = {}
            for t0 in range(0, T, CHT):
                ci = t0 // CHT
                ship = trans_on and (ci % 2 == 1)
                xc = xp.tile([128, CHT * DA], bf16, tag="xc")
                nc.sync.dma_start(
                    xc[:],
                    x_d[t0 * 128 : (t0 + CHT) * 128, :].rearrange(
                        "(p j) d -> p (j d)", p=128
                    ),
                )
                if ship:
                    # host pre-transposed x^T for this whole chunk: [h, p, n]
                    si = ci // 2
                    xtc = xtcp.tile([128, 2 * CHN], bf16, tag="xtc")
                    nc.sync.dma_start(
                        xtc[:, :].rearrange("p (h n) -> p h n", h=2),
                        xt_d[:, :, si * CHN : (si + 1) * CHN].rearrange(
                            "h p n -> p h n"
                        ),
                    )
                s_ps = ps_s.tile([128, CHT], f32, tag="sps")
                for st in range(t0, t0 + CHT, STT):
                    if not score_on:
                        break
                    so = (st - t0) // STT
                    if ship:
                        mlp_rhs = [
                            xtc[:, h * CHN + so * 512 : h * CHN + (so + 1) * 512]
                            for h in (0, 1)
                        ]
                    elif trans_on:
                        # --- on-chip transpose of 4 tiles via PE ---
                        xt_s = xtp.tile([128, 2 * STT * 128], bf16, tag="xts")
                        # one PSUM bank holds all 8 transposed 128x128 panels
                        # of the supertile, laid out [c, q] to match xt_s; a
                        # single wide copy moves it to SBUF (fewer sem hops)
                        xt_ps = ps_xt.tile([128, 2 * STT * 128], bf16, tag="xtps")
                        xt_pv = xt_ps[:, :].rearrange("p (c q) -> p c q", c=2)
                        for jj in range(STT):
                            j = (st - t0) + jj
                            for c in (0, 1):
                                nc.tensor.transpose(
                                    xt_pv[:, c, jj * 128 : jj * 128 + 128],
                                    xc[:, j * DA + c * 128 : j * DA + c * 128 + 128],
                                    id_s[:],
                                )
                        nc.vector.tensor_copy(xt_s[:], xt_ps[:])
                        mlp_rhs = [xt_s[:, 0:512], xt_s[:, 512:1024]]
                    else:
                        mlp_rhs = [xdum_s[:, 0:512], xdum_s[:, 512:1024]]
                    # --- MLP: U^T[h, n] over the 512-node supertile ---
                    u_ps = ps_u.tile([128, 512], f32, tag="ups")
                    nc.tensor.matmul(
                        u_ps[:], w1_s[:, 0:128], mlp_rhs[0], start=True, stop=False
                    )
                    nc.tensor.matmul(
                        u_ps[:], w1_s[:, 128:256], mlp_rhs[1],
                        start=False, stop=True,
                    )
                    tt_s = ttp.tile([128, 512], bf16, tag="tts")
                    nc.scalar.activation(tt_s[:], u_ps[:], AFT.Tanh, bias=b1_s[:])
                    for j in range(STT):
                        col = (st - t0) + j
                        nc.tensor.matmul(
                            s_ps[:, col : col + 1],
                            tt_s[:, j * 128 : (j + 1) * 128],
                            w2_s[:],
                            start=True,
                            stop=True,
                        )
                # --- e = exp(s + b2) for the whole 16-tile chunk ---
                e_s = ep.tile([128, CHT], f32, tag="es")
                if score_on:
                    nc.scalar.activation(e_s[:], s_ps[:], AFT.Exp, bias=b2_s[:])
                else:
                    nc.vector.memset(e_s[:], 1.0)
                # --- pooling matmuls ---
                for j in range(CHT):
                    t = t0 + j
                    x_t = xc[:, j * DA : j * DA + DA]
                    for c in (0, 1):
                        if not pool_on:
                            continue
                        if not (first[c] <= t <= last[c]):
                            continue
                        if t == first[c]:
                            acc_t = ps_acc.tile([128, 257], f32, tag="acc")
                            acc[c] = acc_t
                        oe_s = oep.tile([128, GCH], bf16, tag="oes")
                        nc.vector.tensor_scalar(
                            oe_s[:],
                            iota_s[:, c * GCH : (c + 1) * GCH],
                            brel_s[:, t : t + 1],
                            e_s[:, j : j + 1],
                            op0=ALU.is_equal,
                            op1=ALU.mult,
                        )
                        nc.tensor.matmul(
                            acc[c][:, 0 : D + 1],
                            oe_s[:],
                            x_t[:],
                            start=(t == first[c]),
                            stop=(t == last[c]),
                        )
                        if t == last[c]:
                            den = smallp.tile([128, 1], f32, tag="den")
                            nc.vector.tensor_scalar_add(
                                den[:], acc[c][:, D : D + 1], 1e-30
                            )
                            rec = smallp.tile([128, 1], f32, tag="rec")
                            nc.vector.reciprocal(rec[:], den[:])
                            o_s = outp.tile([128, D], f32, tag="os")
                            nc.vector.tensor_scalar_mul(o_s[:], acc[c][:, 0:D], rec[:])
                            nc.sync.dma_start(out_d[c * GCH : (c + 1) * GCH, :], o_s[:])

        if repeat == 1:
            body()
        else:
            with tc.For_i(0, repeat, 1) as _i:
                body(_i)
    nc.compile()
    return nc


def _get_program(C, t_lo1, t_hi0, repeat=1, ablate=""):
    key = (C, t_lo1, t_hi0, repeat, ablate)
    if key not in _prog_cache:
        _prog_cache[key] = _build_program(C, t_lo1, t_hi0, repeat, ablate)
    return _prog_cache[key]


def _prep_inputs(x, batch, W1, b1, W2, b2):
    """Host-side sharding: split nodes at graph boundaries, permute each
    2048-node chunk so DMA descriptors are fully contiguous (slot 16p+j ->
    partition p, tile j), bake the ones column, cast to bf16."""
    import ml_dtypes

    bf16 = ml_dtypes.bfloat16
    x = np.ascontiguousarray(x, dtype=np.float32)
    batch = np.asarray(batch)
    W1 = np.ascontiguousarray(W1, dtype=np.float32)

    bounds = np.searchsorted(batch, np.arange(0, B_FULL + 1, B_LOC))  # [9]
    mids = np.searchsorted(batch, np.arange(GCH, B_FULL, B_LOC))  # chunk mids [8]
    n_k = np.diff(bounds)
    cap = int(n_k.max())
    C = max(67584, ((cap + CHN - 1) // CHN) * CHN)
    T = C // 128

    b_rel = mids - bounds[:-1]
    # with chunk-permuted node order, a graph boundary at node m spreads the
    # last partial chunk's nodes across all 16 of its tiles
    t_lo1 = int(min(b_rel // CHN)) * CHT
    t_hi0 = int(max((b_rel + CHN - 1) // CHN)) * CHT
    t_lo1 = max(0, min(t_lo1, T))
    t_hi0 = max(1, min(t_hi0, T))

    shared = {
        "w1": W1.reshape(2, 128, H).astype(bf16),
        "b1": np.ascontiguousarray(b1, dtype=np.float32).reshape(H, 1),
        "w2": np.ascontiguousarray(W2, dtype=np.float32).reshape(H, 1).astype(bf16),
        "b2": np.full((128, 1), float(np.asarray(b2).reshape(-1)[0]), np.float32),
        "ident": np.eye(128, dtype=np.float32).astype(bf16),
        "iota": np.broadcast_to(
            np.arange(B_LOC, dtype=np.float32), (128, B_LOC)
        ).copy(),
    }
    in_maps = []
    for k in range(NCORES):
        s, e = int(bounds[k]), int(bounds[k + 1])
        n = e - s
        xk = np.zeros((C, DA), bf16)
        xk[:n, 0:D] = x[s:e].astype(bf16)
        xk[:n, D] = bf16(1.0)
        # pre-transposed x^T for odd chunks: xt[h, p', n] where within chunk
        # ci the on-chip column n = j*128 + p holds x row ci*CHN + 16p + j
        TC = C // CHN
        nship = TC // 2
        xt = np.zeros((2, 128, max(nship, 1) * CHN), bf16)
        for si in range(nship):
            ci = 2 * si + 1
            slab = xk[ci * CHN : (ci + 1) * CHN, 0:D]  # rows: 16p + j
            # [16p+j, d] -> [d, j*128 + p]
            arr = slab.reshape(128, CHT, D).transpose(2, 1, 0).reshape(D, CHN)
            xt[:, :, si * CHN : (si + 1) * CHN] = arr.reshape(2, 128, CHN)
        br = np.full((C,), PAD_SENTINEL, np.float32)
        br[:n] = batch[s:e].astype(np.float32) - k * B_LOC
        # permute within each chunk: new_row[16p + j] = old_row[?]; we need
        # slot r=16p+j to hold the node that tile j, partition p should see.
        # Simplest consistent choice: node (c*2048 + r) stays at slot r, and
        # brel is laid out to match: brel[p, 16c + j] = br[c*2048 + 16p + j].
        brt = br.reshape(T // CHT, 128, CHT)  # [c, p, j]
        brel = np.ascontiguousarray(
            brt.transpose(1, 0, 2).reshape(128, T)
        )  # [p, 16c + j]
        in_maps.append({"x": xk, "xt": xt, "brel": brel, **shared})
    return in_maps, C, t_lo1, t_hi0


def kernel(x, batch, W1, b1, W2, b2):
    from concourse.bass_utils import run_bass_kernel_spmd

    in_maps, C, t_lo1, t_hi0 = _prep_inputs(x, batch, W1, b1, W2, b2)
    nc = _get_program(C, t_lo1, t_hi0)
    res = run_bass_kernel_spmd(nc, in_maps, list(range(NCORES)))
    out = np.concatenate([res.results[k]["out"] for k in range(NCORES)], axis=0)
    return np.ascontiguousarray(out, dtype=np.float32)
